# revision 5
# baseline (speedup 1.0000x reference)
"""Trainium2 Bass kernel for ArcDecoder pair scoring.

Reference computation (N=768 nodes, H=128 features):
    pairs (i, j), i != j:  out[i,j] = W2 @ relu(W1a @ z_i + W1b @ z_j + b1) + b2

Device-side work is only the O(N^2 * H) part:
    H_a = relu(Bt + Abias[:, a]);  out[a, :] = W2 @ H_a
with Abias = W1a @ z.T + b1 and Bt = W1b @ z.T precomputed on host
(0.5% of the FLOPs); b2 is added host-side during assembly.

Per-core layout (core c owns i-rows [96c, 96c+96)):
  Bt [128, 768] bf16 (replicated, one fat-row DMA) and
  AB [128, 97] f32 = [Abias slice | W2 row] (second queue).
  S (zero-padded W2 stationary) is built on-chip with no input
  dependency (memset zero + broadcast copies), so round-0 matmuls
  are gated only by the first H tile.
  H tiles split 68:28 over DVE/ACT; PSUM generations are asymmetric
  (24 + 8 rounds) with generation 0's eviction deferred into
  generation 1; output is group-major [3, 32, 768] so every output
  DMA is one contiguous block (the host deinterleaves for free).
"""

import numpy as np
import ml_dtypes

import concourse.bass as bass
import concourse.tile as tile
from concourse import bacc, mybir
from concourse.bass_utils import run_bass_kernel_spmd

N = 768
H = 128
NCORES = 8
ROWS = N // NCORES          # 96 i-rows per core
NGRP = 3                    # PE column groups (PSUM partitions 32g..32g+31)
RND = ROWS // NGRP          # 32 rounds; round r, group g handles a = 3r + g
GRND0 = 24                  # rounds in PSUM generation 0
GRND1 = RND - GRND0         # 8 rounds in generation 1 (short tail)
QMAX = GRND0
HALF = N // 2               # 384, PSUM bank limit for f32 is 512


_F32 = mybir.dt.float32
_BF16 = mybir.dt.bfloat16

_cache = {}


def _build():
    nc = bacc.Bacc(
        "TRN2",
        target_bir_lowering=False,
        debug=False,
        enable_asserts=False,
        num_devices=NCORES,
    )

    ab_d = nc.dram_tensor("AB", [H, ROWS + 1], _F32, kind="ExternalInput")
    bt_d = nc.dram_tensor("Bt", [H, N], _BF16, kind="ExternalInput")
    out_d = nc.dram_tensor("out", [NGRP, RND, N], _F32, kind="ExternalOutput")

    relu = mybir.ActivationFunctionType.Relu
    copyf = mybir.ActivationFunctionType.Copy
    add_op = mybir.AluOpType.add
    max_op = mybir.AluOpType.max

    with tile.TileContext(nc) as tc:
        with (
            tc.tile_pool(name="const", bufs=1) as cpool,
            tc.tile_pool(name="hpool", bufs=8) as hpool,
            tc.tile_pool(name="opool", bufs=2) as opool,
            tc.tile_pool(name="psum", bufs=2, space=bass.MemorySpace.PSUM) as pspool,
        ):
            # ACT spline-table prewarm (Relu + Copy) so the one-time
            # ACT_TABLE_LOAD overlaps the input DMAs.
            scratch = cpool.tile([1, 8], _F32, tag="scratch")
            nc.gpsimd.memset(scratch[:], 0.0)
            nc.scalar.activation(scratch[:], scratch[:], relu)
            nc.scalar.activation(scratch[:], scratch[:], copyf)

            # fat 1536B rows on one queue beat a column-split (the two
            # HWDGE queues share the same DMA engines / bandwidth)
            bt_sb = cpool.tile([H, N], _BF16)
            nc.sync.dma_start(bt_sb[:], bt_d[:])
            ab_sb = cpool.tile([H, ROWS + 1], _F32)
            nc.scalar.dma_start(ab_sb[:], ab_d[:])
            bt = bt_sb[:]

            # Zero-padded W2 stationary S[k, q, q] = W2[0, k], else 0.
            # Zero-fill has no input dependency (memset + broadcast copy on
            # the otherwise-idle ACT queue); only the tiny diagonal write
            # waits for the input DMA.
            zcol = cpool.tile([H, 1], _BF16, tag="zcol")
            nc.gpsimd.memset(zcol[:], 0.0)
            S_sb = cpool.tile([H, QMAX, 32], _BF16)
            S_flat = S_sb[:].rearrange("k q m -> k (q m)")
            nc.scalar.activation(S_flat, zcol[:].broadcast_to([H, QMAX * 32]), copyf)
            diag = S_flat[:, 0 : (QMAX - 1) * 33 + 1 : 33]
            nc.vector.tensor_copy(diag, ab_sb[:, ROWS : ROWS + 1].broadcast_to([H, QMAX]))

            gen_rounds = (GRND0, GRND1)
            gen_base = (0, GRND0)
            deferred_evict = []

            def emit_evict(t):
                grnd = gen_rounds[t]
                ps, ot = deferred_evict.pop(0)
                for h in range(2):
                    if h == 0:
                        nc.scalar.activation(
                            ot[:, h * HALF : (h + 1) * HALF], ps[h][:], copyf
                        )
                    else:
                        nc.vector.tensor_copy(
                            ot[:, h * HALF : (h + 1) * HALF], ps[h][:]
                        )
                for g in range(NGRP):
                    eng = nc.sync if (t == 0 or g != 1) else nc.scalar
                    eng.dma_start(
                        out_d.ap()[g, gen_base[t] : gen_base[t] + grnd],
                        ot[32 * g : 32 * g + grnd, :],
                    )

            for t in range(2):
                grnd = gen_rounds[t]
                ps = [
                    pspool.tile([ROWS, HALF], _F32, tag=f"ps{h}", name=f"ps{h}_{t}")
                    for h in range(2)
                ]
                for q in range(grnd):
                    r = gen_base[t] + q
                    hts = [None] * NGRP
                    # DVE tensor_scalar ~414ns/tile, ACT activation ~860ns;
                    # 68:28 split (ACT also does the eviction copies); the
                    # all-DVE round 27 coincides with gen 0's ACT eviction.
                    engs = (
                        ("dve", "dve", "dve")
                        if r in (6, 14, 22, 27)
                        else ("dve", "dve", "act")
                    )
                    order = sorted(range(NGRP), key=lambda g: engs[g] == "dve")
                    for g in order:
                        a = NGRP * r + g
                        ht = hpool.tile([H, N], _BF16, tag="H", name=f"h{a}")
                        if engs[g] == "dve":
                            nc.vector.tensor_scalar(
                                ht[:], bt, ab_sb[:, a : a + 1], 0.0,
                                add_op, max_op,
                            )
                        else:
                            nc.scalar.activation(
                                ht[:], bt, relu,
                                bias=ab_sb[:, a : a + 1], scale=1.0,
                            )
                        hts[g] = ht
                    first = q == 0
                    last = q == grnd - 1
                    for h in range(2):
                        for g in range(NGRP):
                            nc.tensor.matmul(
                                ps[h][32 * g : 32 * g + 32, :],
                                S_sb[:, q, :],
                                hts[g][:, h * HALF : (h + 1) * HALF],
                                start=first,
                                stop=last,
                            )
                    # gen 0's eviction is emitted a few rounds into gen 1 so
                    # the copies don't displace boundary-round H tiles.
                    if t == 1 and q == 3:
                        emit_evict(0)
                ot = opool.tile([ROWS, N], _F32, tag="ot", name=f"ot{t}")
                deferred_evict.append((ps, ot))
            emit_evict(1)

    nc.compile()
    return nc


def _get_nc():
    if "nc" not in _cache:
        _cache["nc"] = _build()
    return _cache["nc"]


def _prep_in_maps(z, W1, b1, W2, b2):
    z = np.asarray(z, np.float32)
    W1 = np.asarray(W1, np.float32)
    b1 = np.asarray(b1, np.float32)
    W2 = np.asarray(W2, np.float32)

    bf = ml_dtypes.bfloat16
    zT = z.T  # [H, N]
    abias = W1[:, :H] @ zT + b1[:, None]            # [H, N] f32
    bt = (W1[:, H:] @ zT).astype(bf)                # [H, N] bf16

    in_maps = []
    for c in range(NCORES):
        ab = np.empty((H, ROWS + 1), np.float32)
        ab[:, :ROWS] = abias[:, c * ROWS : (c + 1) * ROWS]
        ab[:, ROWS] = W2[0]
        in_maps.append({"AB": ab, "Bt": bt})
    return in_maps


def _assemble(results, b2):
    full = np.empty((N, N), np.float32)
    for c in range(NCORES):
        o = np.asarray(results[c]["out"], np.float32)  # [3, 32, 768] group-major
        blk = full[c * ROWS : (c + 1) * ROWS]
        for g in range(NGRP):
            blk[g::NGRP] = o[g]
    full += b2
    mask = ~np.eye(N, dtype=bool)
    return full[mask]  # pair-major order: i-major, j ascending, j != i


def run(z, W1, b1, W2, b2, trace=False, tmpdir=None):
    nc = _get_nc()
    in_maps = _prep_in_maps(z, W1, b1, W2, b2)
    res = run_bass_kernel_spmd(
        nc, in_maps, core_ids=list(range(NCORES)), trace=trace, tmpdir=tmpdir
    )
    return _assemble(res.results, float(np.asarray(b2, np.float32)[0])), res


def kernel(z, W1, b1, W2, b2):
    out, _ = run(z, W1, b1, W2, b2, trace=False)
    return out
